# revision 10
# baseline (speedup 1.0000x reference)
"""CycleConsistencyLoss on 8 Trainium2 NeuronCores (Bass/Tile, SPMD data-parallel).

Math (per batch, clip [M,D], sent [N,D], prefix masks):
  soft_nn(src,tgt): w = softmax_j(-dist(src_i,tgt_j) masked); nn = w @ tgt
  dist = (|s|^2+|t|^2-2 s.t)/D; softmax shift-invariance =>
  w[i,j] prop exp((2 s_i.t_j - |t_j|^2)/D) * mask_j
  index_nn = sum_u u*beta / sum_u beta over tgt2 = src embeddings
  loss_c = mean_b sum_i (index_nn[i]-i)^2 * mask_i / len_b

fp8 e4m3 design (validated vs f64 reference at ~1e-5 rel err):
  All embedding operands pre-quantized e4m3 on host; matmuls use fp8
  DoubleRow perf mode where the layout allows:
   S1  scores: normal-mode fp8 [128,*] (LD hides under stream)
   S2  nn acc: lhsT [128,2,128] over j-block PAIRS (K=256)    -> 0.25
   D   beta:   normal-mode fp8 [128,*]                        -> 1.0
   FL  den/num: lhsT [128,2,4] over u-block pairs             -> 0.25-ish
  index weights for FL are digit-split u = 64*d1 + 16*d2 + d3 (each digit
  e4m3-exact); recombined in f32 in the final phase.
  exp runs on TWO engines, split per score tile:
   ACT: exp activation, bias=-|t|^2/D(+pen), scale=1/A8, fp8 out
   DVE: Schraudolph bit-hack: uint8 = max(psum + biasbits, 0) where psum is
        pre-scaled by A8*2/D via the host embedding scaling; uint8 bit
        pattern IS the e4m3 weight. Masked rows -> bits<0 -> clamp 0.
  Odd block counts are padded to even: the pad block's scores get the
  masked-row bias (pen) so its weights underflow to exactly 0 in fp8.
  th staging is pre-zeroed via a DMA'd constant (den stripe=1.0) so units
  with se<=512 leave benign values in the unused half.
"""
import sys

sys.path.insert(0, "/opt/trn_rl_repo")

import numpy as np
import ml_dtypes

import concourse.bass as bass
import concourse.tile as tile
from concourse import bacc, mybir
from concourse.bass_utils import run_bass_kernel_spmd

F32 = mybir.dt.float32
FP8 = mybir.dt.float8e4
U8 = mybir.dt.uint8
EXP = mybir.ActivationFunctionType.Exp
ALU = mybir.AluOpType
DR = mybir.MatmulPerfMode.DoubleRow
FP16 = mybir.dt.float16

B, M, N, D = 32, 1024, 1024, 128
NB = M // 128
NCORES = 8
SLOTS = B // NCORES  # 4
NUNITS = 2 * SLOTS
PEN = -20.0
A8 = 8.0 / np.log(2.0)       # e4m3 bits per e-fold
MAGIC = 56.5                  # 7*8 exponent bias + 0.5 round
ALPHA1 = np.sqrt(2.0 * A8 / D)  # S1 per-side embedding scale
ALPHA2 = 2.0 * A8 / D           # D-stage lhsT scale
E4 = ml_dtypes.float8_e4m3

# exp engine split: 'A' = ACT exact exp, 'D' = DVE bit-hack
EXP_PATTERN = "A"
TH_COPY_ENGINE = "A"

_PROGRAM_CACHE = {}
LAST_RESULT = None


def _chunks(ext):
    if ext <= 512:
        return [(0, ext)]
    return [(0, 512), (512, ext - 512)]


def _emit(nc, tc, ctx, io, plans):
    const = ctx.enter_context(tc.tile_pool(name="const", bufs=1))
    s1p = ctx.enter_context(tc.tile_pool(name="s1p", bufs=2))
    dp = ctx.enter_context(tc.tile_pool(name="dp", bufs=2))
    xwp = ctx.enter_context(tc.tile_pool(name="xwp", bufs=2))
    bp = ctx.enter_context(tc.tile_pool(name="bp", bufs=2))
    etp = ctx.enter_context(tc.tile_pool(name="etp", bufs=2))
    btp = ctx.enter_context(tc.tile_pool(name="btp", bufs=2))
    nnsp = ctx.enter_context(tc.tile_pool(name="nnsp", bufs=2))
    rrp = ctx.enter_context(tc.tile_pool(name="rrp", bufs=2))
    bcp = ctx.enter_context(tc.tile_pool(name="bcp", bufs=2))
    fin = ctx.enter_context(tc.tile_pool(name="fin", bufs=1))

    ps_big = ctx.enter_context(tc.tile_pool(name="ps_big", bufs=3, space="PSUM"))
    ps_nn = ctx.enter_context(tc.tile_pool(name="ps_nn", bufs=1, space="PSUM"))

    thin4w = const.tile([128, NB, 16], FP8, tag="thin4w")
    iota32 = const.tile([32, 256], F32, tag="iota32")
    masks32 = const.tile([32, 256], F32, tag="masks32")
    rlens32 = const.tile([32, 1], F32, tag="rlens32")
    # th staging pre-zeroed (den stripe 1.0) so unwritten halves stay benign
    thstage = const.tile([4, NUNITS, 2, 512], F32, tag="thstage")

    def load_consts():
        nc.sync.dma_start(out=thin4w, in_=io["thin4w"])
        nc.scalar.dma_start(out=iota32, in_=io["iota32"])
        nc.sync.dma_start(out=masks32, in_=io["masks32"])
        nc.scalar.dma_start(out=rlens32, in_=io["rlens32"])
        nc.scalar.dma_start(out=thstage, in_=io["zfill"])

    exp_cycle = [0]

    def exp_step(u, ty, idx, big, dst8, se):
        eng = EXP_PATTERN[exp_cycle[0] % len(EXP_PATTERN)]
        exp_cycle[0] += 1
        bias = u["bias"]
        if eng == "A":
            nc.scalar.activation(dst8, big[:, 0:se], EXP,
                                 bias=bias[:, 0, ty, idx:idx + 1],
                                 scale=float(1.0 / A8))
        else:
            nc.vector.tensor_scalar(out=dst8.bitcast(U8), in0=big[:, 0:se],
                                    scalar1=bias[:, 1, ty, idx:idx + 1],
                                    scalar2=0.0, op0=ALU.add, op1=ALU.max)

    slot_tiles = {}

    def get_slot(s):
        if s in slot_tiles:
            return slot_tiles[s]
        bias = bp.tile([128, 2, 2, NB], F32, tag="bias", name=f"bias{s}")
        nc.sync.dma_start(out=bias, in_=io["bias"][s])
        s1t = s1p.tile([128, 2, 1024], FP8, tag="s1t", name=f"s1t{s}")
        nc.sync.dma_start(out=s1t[:, 0], in_=io["emb1"][s, :, 0])
        nc.scalar.dma_start(out=s1t[:, 1], in_=io["emb1"][s, :, 1])
        dt = dp.tile([128, 2, 1024], FP16, tag="dt", name=f"dt{s}")
        nc.scalar.dma_start(out=dt[:, 0], in_=io["emb2"][s, :, 0])
        nc.sync.dma_start(out=dt[:, 1], in_=io["emb2"][s, :, 1])
        xw = xwp.tile([128, 2, NB, 128], FP8, tag="xw", name=f"xw{s}")
        nc.scalar.dma_start(out=xw[:, 0], in_=io["xw"][s, :, 0])
        nc.sync.dma_start(out=xw[:, 1], in_=io["xw"][s, :, 1])
        t = {"s1c": s1t[:, 0], "s1s": s1t[:, 1],
             "dc": dt[:, 0], "ds": dt[:, 1],
             "xwa": xw[:, 0], "xwb": xw[:, 1], "bias": bias}
        slot_tiles[s] = t
        return t

    def s1_step(u, tb):
        big = ps_big.tile([128, 1024], F32, tag="big", name=f"s1_{u['k']}_{tb}")
        se = u["se"]
        for off, w in _chunks(se):
            nc.tensor.matmul(big[:, off:off + w],
                             lhsT=u["X"][:, 128 * tb:128 * (tb + 1)],
                             rhs=u["Y"][:, off:off + w],
                             start=True, stop=True)
        exp_step(u, u["ty_t"], tb, big, u["et"][:, tb, 0:se], se)

    def s2_c_phase(u, nxt):
        """nn accumulation over j-block pairs (K=256 DoubleRow) into a single
        [128,1024] PSUM tile; row 0 = den. S1 steps of the next unit are
        interleaved to keep the exp engines fed."""
        k, se, Etile = u["k"], u["se"], u["et"]
        npairs = u["nb_t"] // 2
        chunks = _chunks(se)
        n_s1 = nxt["nb_t"] if nxt is not None else 0
        n_res = min(3, n_s1)
        n_pre = n_s1 - n_res
        total_nn = npairs * len(chunks)
        s1_done = 0
        nn_done = 0
        nnp = ps_nn.tile([128, 1024], F32, tag="nn", name=f"nn_{k}")
        for pb in range(npairs):
            for off, w in chunks:
                nc.tensor.matmul(nnp[:, off:off + w],
                                 lhsT=u["xw"][:, 2 * pb:2 * pb + 2, :],
                                 rhs=Etile[:, 2 * pb:2 * pb + 2, off:off + w],
                                 start=pb == 0, stop=pb == npairs - 1,
                                 perf_mode=DR)
                nn_done += 1
                while n_pre and s1_done < (n_pre * nn_done) // total_nn:
                    s1_step(nxt, s1_done)
                    s1_done += 1
        while s1_done < n_pre:
            s1_step(nxt, s1_done)
            s1_done += 1
        rr = rrp.tile([1, 1024], F32, tag="rr")
        nc.vector.reciprocal_approx_fast(out=rr[:, 0:se], in_=nnp[0:1, 0:se])
        bc = bcp.tile([128, 1024], F32, tag="bc")
        nc.gpsimd.partition_broadcast(bc[:, 0:se], rr[:, 0:se])
        nns = nnsp.tile([128, 1024], FP16, tag="nns", name=f"nns_{k}")
        nc.scalar.activation(nns[:, 0:se], nnp[:, 0:se],
                             mybir.ActivationFunctionType.Copy)
        while s1_done < n_s1:
            s1_step(nxt, s1_done)
            s1_done += 1
        u["bc"] = bc
        return nns

    def d_step(u, ub):
        se = u["se"]
        big2 = ps_big.tile([128, 1024], F32, tag="big", name=f"d_{u['k']}_{ub}")
        for off, w in _chunks(se):
            nc.tensor.matmul(big2[:, off:off + w],
                             lhsT=u["Dlhs"][:, 128 * ub:128 * (ub + 1)],
                             rhs=u["nns"][:, off:off + w], start=True, stop=True)
        nc.vector.scalar_tensor_tensor(u["bt"][:, ub, 0:se].bitcast(U8),
                                       in0=big2[:, 0:se], scalar=0.0,
                                       in1=u["bc"][:, 0:se],
                                       op0=ALU.max, op1=ALU.mult)

    def d_flush(u, upb):
        nupairs = u["nb_u"] // 2
        for qi, (off, w) in enumerate(_chunks(u["se"])):
            nc.tensor.matmul(u["th"][0:4, qi, 0:w],
                             lhsT=thin4w[:, 2 * upb:2 * upb + 2, 0:4],
                             rhs=u["bt"][:, 2 * upb:2 * upb + 2, off:off + w],
                             start=upb == 0, stop=upb == nupairs - 1,
                             perf_mode=DR)

    def even(x):
        return x + (x & 1)

    units = []
    for s in range(SLOTS):
        cb, sb = plans[s]
        t = get_slot(s)
        # cycle0: clip->sent->clip; tgt blocks = sent, D/u blocks = clip
        units.append(dict(k=2 * s, nb_t=even(sb), nb_u=even(cb), se=cb * 128,
                          X=t["s1s"], Y=t["s1c"], ty_t=1, ty_u=0,
                          xw=t["xwa"], Dlhs=t["dc"], bias=t["bias"]))
        units.append(dict(k=2 * s + 1, nb_t=even(cb), nb_u=even(sb), se=sb * 128,
                          X=t["s1c"], Y=t["s1s"], ty_t=0, ty_u=1,
                          xw=t["xwb"], Dlhs=t["ds"], bias=t["bias"]))

    load_consts()

    # prologue: S1 of unit 0
    units[0]["et"] = etp.tile([128, NB, 1024], FP8, tag="et", name="et0")
    for tb in range(units[0]["nb_t"]):
        s1_step(units[0], tb)

    acc_tiles = [fin.tile([32, 256], F32, tag=f"acc{c}", name=f"acc{c}")
                 for c in range(4)]

    LAGP = 1
    for j, u in enumerate(units):
        nxt = units[j + 1] if j + 1 < len(units) else None
        if nxt is not None:
            nxt["et"] = etp.tile([128, NB, 1024], FP8, tag="et",
                                 name=f"et{nxt['k']}")
        u["nns"] = s2_c_phase(u, nxt)
        u["bt"] = btp.tile([128, NB, 1024], FP8, tag="bt", name=f"bt{u['k']}")
        u["th"] = ps_nn.tile([4, 2, 512], F32, tag="nn", name=f"th_{u['k']}")
        nupairs = u["nb_u"] // 2
        for ub in range(u["nb_u"]):
            d_step(u, ub)
            if ub % 2 == 1 and (ub // 2) >= LAGP:
                d_flush(u, ub // 2 - LAGP)
        for upb in range(max(0, nupairs - LAGP), nupairs):
            d_flush(u, upb)
        for qi, (off, w) in enumerate(_chunks(u["se"])):
            if TH_COPY_ENGINE == "A":
                nc.scalar.copy(out=thstage[:, u["k"], qi, 0:w],
                               in_=u["th"][:, qi, 0:w])
            else:
                nc.vector.tensor_copy(out=thstage[:, u["k"], qi, 0:w],
                                      in_=u["th"][:, qi, 0:w])
        k = u["k"]
        for c in range(4):
            nc.sync.dma_start(out=acc_tiles[c][4 * k:4 * k + 4, :],
                              in_=thstage[c:c + 1, k, :, :])

    # ---- final: batched loss over [32, 256] rows = (unit, qi, seg) ----
    den32, d1, d2, d3 = acc_tiles
    t4 = fin.tile([32, 256], F32, tag="t4")
    nc.vector.scalar_tensor_tensor(t4, in0=d1, scalar=4.0, in1=d2,
                                   op0=ALU.mult, op1=ALU.add)
    num = fin.tile([32, 256], F32, tag="num")
    nc.vector.scalar_tensor_tensor(num, in0=t4, scalar=16.0, in1=d3,
                                   op0=ALU.mult, op1=ALU.add)
    rden = fin.tile([32, 256], F32, tag="rden")
    scr = fin.tile([32, 256], F32, tag="scr")
    nc.vector.reciprocal_approx_accurate(out=rden, in_=den32, scratch=scr)
    idx = fin.tile([32, 256], F32, tag="idx")
    nc.vector.tensor_mul(idx, num, rden)
    ierr = fin.tile([32, 256], F32, tag="ierr")
    nc.vector.tensor_sub(ierr, idx, iota32)
    tmp = fin.tile([32, 256], F32, tag="tmp")
    nc.vector.tensor_mul(tmp, ierr, masks32)
    sq = fin.tile([32, 256], F32, tag="sq")
    sums = fin.tile([32, 1], F32, tag="sums")
    nc.vector.scalar_tensor_tensor(sq, in0=tmp, scalar=1.0, in1=ierr,
                                   op0=ALU.bypass, op1=ALU.mult, accum_out=sums)
    loss = fin.tile([32, 1], F32, tag="loss")
    nc.vector.tensor_mul(loss, sums, rlens32)
    nc.sync.dma_start(out=io["loss32"], in_=loss)


def _build_program(plans):
    key = tuple(plans)
    if key in _PROGRAM_CACHE:
        return _PROGRAM_CACHE[key]
    nc = bacc.Bacc("TRN2", target_bir_lowering=False, debug=False,
                   num_devices=NCORES)
    io = {
        "emb1": nc.dram_tensor("emb1", [SLOTS, 128, 2, 1024], FP8, kind="ExternalInput").ap(),
        "emb2": nc.dram_tensor("emb2", [SLOTS, 128, 2, 1024], FP16, kind="ExternalInput").ap(),
        "xw": nc.dram_tensor("xw", [SLOTS, 128, 2, NB, 128], FP8, kind="ExternalInput").ap(),
        "bias": nc.dram_tensor("bias", [SLOTS, 128, 2, 2, NB], F32, kind="ExternalInput").ap(),
        "thin4w": nc.dram_tensor("thin4w", [128, NB, 16], FP8, kind="ExternalInput").ap(),
        "iota32": nc.dram_tensor("iota32", [32, 256], F32, kind="ExternalInput").ap(),
        "masks32": nc.dram_tensor("masks32", [32, 256], F32, kind="ExternalInput").ap(),
        "rlens32": nc.dram_tensor("rlens32", [32, 1], F32, kind="ExternalInput").ap(),
        "zfill": nc.dram_tensor("zfill", [4, NUNITS, 2, 512], F32, kind="ExternalInput").ap(),
        "loss32": nc.dram_tensor("loss32", [32, 1], F32, kind="ExternalOutput").ap(),
    }
    from contextlib import ExitStack
    with tile.TileContext(nc) as tc:
        with ExitStack() as ctx:
            _emit(nc, tc, ctx, io, plans)
    nc.compile()
    _PROGRAM_CACHE[key] = nc
    return nc


def _q8(x):
    return np.asarray(x, np.float32).astype(E4)


def _host_prep(clip_emb, clip_mask, clip_lens, sent_emb, sent_mask, sent_lens):
    """Sorted batch->(core,slot) assignment, per-slot plans, per-core inputs."""
    cb_all = np.ceil(clip_lens / 128).astype(int)
    sb_all = np.ceil(sent_lens / 128).astype(int)
    order = np.argsort(-(cb_all + sb_all) * 1000 - cb_all)  # big batches first
    plans = []
    assign = {}
    for s in range(SLOTS):
        grp = order[8 * s:8 * s + 8]
        plans.append((int(cb_all[grp].max()), int(sb_all[grp].max())))
        for core, b in enumerate(grp):
            assign[(core, s)] = int(b)

    sq_c = np.einsum("bmd,bmd->bm", clip_emb, clip_emb)
    sq_s = np.einsum("bnd,bnd->bn", sent_emb, sent_emb)
    bias_c = (-sq_c / D + PEN * (1.0 - clip_mask)).astype(np.float32)
    bias_s = (-sq_s / D + PEN * (1.0 - sent_mask)).astype(np.float32)

    u = np.arange(M)
    thin4w = np.zeros((128, NB, 16), E4)
    for ub in range(NB):
        uu = u[ub * 128:(ub + 1) * 128]
        thin4w[:, ub, 0] = 1.0
        thin4w[:, ub, 1] = (uu >> 6).astype(np.float32)
        thin4w[:, ub, 2] = ((uu >> 4) & 3).astype(np.float32)
        thin4w[:, ub, 3] = (uu & 15).astype(np.float32)

    iota32 = np.arange(M, dtype=np.float32).reshape(1, 4, 256)
    iota32 = np.broadcast_to(iota32, (NUNITS, 4, 256)).reshape(32, 256).copy()
    zfill = np.zeros((4, NUNITS, 2, 512), np.float32)
    zfill[0] = 1.0

    in_maps = []
    for core in range(NCORES):
        bs = [assign[(core, s)] for s in range(SLOTS)]
        emb1 = np.zeros((SLOTS, 128, 2, 1024), E4)
        emb2 = np.zeros((SLOTS, 128, 2, 1024), np.float16)
        xw = np.zeros((SLOTS, 128, 2, NB, 128), E4)
        bias = np.zeros((SLOTS, 128, 2, 2, NB), np.float32)
        masks32 = np.zeros((32, 256), np.float32)
        rlens32 = np.zeros((32, 1), np.float32)
        for s, b in enumerate(bs):
            c = clip_emb[b].astype(np.float32)   # [M, D]
            t = sent_emb[b].astype(np.float32)   # [N, D]
            # S1 operands: [p(64), kt, i] = emb[i, kt*64+p] * ALPHA1
            cs = _q8(c * ALPHA1)  # [M, D]
            ts = _q8(t * ALPHA1)
            emb1[s, :, 0] = cs.T
            emb1[s, :, 1] = ts.T
            # D lhsT fp16: row0 = kappa_u (full exp bias in bit units),
            # rows 1..127 = emb[u, e-1]*ALPHA2
            kap_c = MAGIC + A8 * PEN * (1.0 - clip_mask[b]) - A8 * sq_c[b] / D
            kap_s = MAGIC + A8 * PEN * (1.0 - sent_mask[b]) - A8 * sq_s[b] / D
            c2 = np.zeros((128, M), np.float32)
            c2[0, :] = kap_c
            c2[1:, :] = (c[:, :127] * ALPHA2).T
            t2 = np.zeros((128, N), np.float32)
            t2[0, :] = kap_s
            t2[1:, :] = (t[:, :127] * ALPHA2).T
            emb2[s, :, 0] = c2.astype(np.float16)
            emb2[s, :, 1] = t2.astype(np.float16)
            # S2 lhsT: [p, tb, e]: e=0 -> 1 else tgt[tb*128+p, e-1]
            xa = np.zeros((128, NB, 128), np.float32)
            xa[:, :, 0] = 1.0
            xa[:, :, 1:] = _q8(t[:, :127]).astype(np.float32).reshape(NB, 128, 127).transpose(1, 0, 2)
            xb = np.zeros((128, NB, 128), np.float32)
            xb[:, :, 0] = 1.0
            xb[:, :, 1:] = _q8(c[:, :127]).astype(np.float32).reshape(NB, 128, 127).transpose(1, 0, 2)
            xw[s, :, 0] = xa.astype(E4)
            xw[s, :, 1] = xb.astype(E4)
            # biases: [p, btype(exact,bits), which(c,s), tb]
            bias[s, :, 0, 0] = bias_c[b].reshape(NB, 128).T
            bias[s, :, 0, 1] = bias_s[b].reshape(NB, 128).T
            bias[s, :, 1, 0] = (A8 * bias_c[b] + MAGIC).reshape(NB, 128).T
            bias[s, :, 1, 1] = (A8 * bias_s[b] + MAGIC).reshape(NB, 128).T
            # final-phase rows r = 4k + 2*qi + seg
            for cyc, (msk, ln) in enumerate(((clip_mask[b], clip_lens[b]),
                                             (sent_mask[b], sent_lens[b]))):
                k = 2 * s + cyc
                masks32[4 * k:4 * k + 4] = msk.reshape(4, 256)
                rlens32[4 * k:4 * k + 4] = 1.0 / ln
        in_maps.append({
            "emb1": emb1, "emb2": emb2, "xw": xw, "bias": bias,
            "thin4w": thin4w, "iota32": iota32, "masks32": masks32,
            "rlens32": rlens32, "zfill": zfill,
        })
    return in_maps, assign, plans


def kernel(clip_emb, clip_mask, clip_lens, sent_emb, sent_mask, sent_lens):
    global LAST_RESULT
    clip_emb = np.asarray(clip_emb, np.float32)
    sent_emb = np.asarray(sent_emb, np.float32)
    clip_mask = np.asarray(clip_mask, np.float32)
    sent_mask = np.asarray(sent_mask, np.float32)
    clip_lens = np.asarray(clip_lens, np.float32)
    sent_lens = np.asarray(sent_lens, np.float32)

    in_maps, _, plans = _host_prep(clip_emb, clip_mask, clip_lens,
                                   sent_emb, sent_mask, sent_lens)
    nc = _build_program(plans)
    res = run_bass_kernel_spmd(nc, in_maps, list(range(NCORES)))
    LAST_RESULT = res

    rows = np.stack([res.results[c]["loss32"].reshape(32) for c in range(NCORES)])
    per_unit = rows.reshape(NCORES, NUNITS, 4).sum(axis=2)
    clip_loss = per_unit[:, 0::2].mean()
    sent_loss = per_unit[:, 1::2].mean()
    return (np.float32(clip_loss), np.float32(sent_loss))


# revision 11
# speedup vs baseline: 1.0625x; 1.0625x over previous
"""CycleConsistencyLoss on 8 Trainium2 NeuronCores (Bass/Tile, SPMD data-parallel).

Math (per batch, clip [M,D], sent [N,D], prefix masks):
  soft_nn(src,tgt): w = softmax_j(-dist(src_i,tgt_j) masked); nn = w @ tgt
  dist = (|s|^2+|t|^2-2 s.t)/D; softmax shift-invariance =>
  w[i,j] prop exp((2 s_i.t_j - |t_j|^2)/D) * mask_j
  index_nn = sum_u u*beta / sum_u beta over tgt2 = src embeddings
  loss_c = mean_b sum_i (index_nn[i]-i)^2 * mask_i / len_b

fp8 e4m3 design (validated vs f64 reference at ~1e-5 rel err):
  All embedding operands pre-quantized e4m3 on host; matmuls use fp8
  DoubleRow perf mode where the layout allows:
   S1  scores: normal-mode fp8 [128,*] (LD hides under stream)
   S2  nn acc: lhsT [128,2,128] over j-block PAIRS (K=256)    -> 0.25
   D   beta:   normal-mode fp8 [128,*]                        -> 1.0
   FL  den/num: lhsT [128,2,4] over u-block pairs             -> 0.25-ish
  index weights for FL are digit-split u = 64*d1 + 16*d2 + d3 (each digit
  e4m3-exact); recombined in f32 in the final phase.
  exp runs on TWO engines, split per score tile:
   ACT: exp activation, bias=-|t|^2/D(+pen), scale=1/A8, fp8 out
   DVE: Schraudolph bit-hack: uint8 = max(psum + biasbits, 0) where psum is
        pre-scaled by A8*2/D via the host embedding scaling; uint8 bit
        pattern IS the e4m3 weight. Masked rows -> bits<0 -> clamp 0.
  Odd block counts are padded to even: the pad block's scores get the
  masked-row bias (pen) so its weights underflow to exactly 0 in fp8.
  th staging is pre-zeroed via a DMA'd constant (den stripe=1.0) so units
  with se<=512 leave benign values in the unused half.
"""
import sys

sys.path.insert(0, "/opt/trn_rl_repo")

import numpy as np
import ml_dtypes

import concourse.bass as bass
import concourse.tile as tile
from concourse import bacc, mybir
from concourse.bass_utils import run_bass_kernel_spmd

F32 = mybir.dt.float32
FP8 = mybir.dt.float8e4
U8 = mybir.dt.uint8
EXP = mybir.ActivationFunctionType.Exp
ALU = mybir.AluOpType
DR = mybir.MatmulPerfMode.DoubleRow
FP16 = mybir.dt.float16

B, M, N, D = 32, 1024, 1024, 128
NB = M // 128
NCORES = 8
SLOTS = B // NCORES  # 4
NUNITS = 2 * SLOTS
PEN = -20.0
A8 = 8.0 / np.log(2.0)       # e4m3 bits per e-fold
MAGIC = 56.5                  # 7*8 exponent bias + 0.5 round
ALPHA1 = np.sqrt(2.0 * A8 / D)  # S1 per-side embedding scale
ALPHA2 = 2.0 * A8 / D           # D-stage lhsT scale
E4 = ml_dtypes.float8_e4m3

# exp engine split: 'A' = ACT exact exp, 'D' = DVE bit-hack
EXP_PATTERN = "ADADADA"
TH_COPY_ENGINE = "A"

_PROGRAM_CACHE = {}
LAST_RESULT = None


def _chunks(ext):
    if ext <= 512:
        return [(0, ext)]
    return [(0, 512), (512, ext - 512)]


def _emit(nc, tc, ctx, io, plans):
    const = ctx.enter_context(tc.tile_pool(name="const", bufs=1))
    s1p = ctx.enter_context(tc.tile_pool(name="s1p", bufs=2))
    dp = ctx.enter_context(tc.tile_pool(name="dp", bufs=2))
    xwp = ctx.enter_context(tc.tile_pool(name="xwp", bufs=2))
    bp = ctx.enter_context(tc.tile_pool(name="bp", bufs=2))
    etp = ctx.enter_context(tc.tile_pool(name="etp", bufs=2))
    btp = ctx.enter_context(tc.tile_pool(name="btp", bufs=2))
    nnsp = ctx.enter_context(tc.tile_pool(name="nnsp", bufs=2))
    rrp = ctx.enter_context(tc.tile_pool(name="rrp", bufs=2))
    bcp = ctx.enter_context(tc.tile_pool(name="bcp", bufs=2))
    fin = ctx.enter_context(tc.tile_pool(name="fin", bufs=1))

    ps_big = ctx.enter_context(tc.tile_pool(name="ps_big", bufs=3, space="PSUM"))
    ps_nn = ctx.enter_context(tc.tile_pool(name="ps_nn", bufs=1, space="PSUM"))

    thin4w = const.tile([128, NB, 16], FP8, tag="thin4w")
    iota32 = const.tile([32, 256], F32, tag="iota32")
    masks32 = const.tile([32, 256], F32, tag="masks32")
    rlens32 = const.tile([32, 1], F32, tag="rlens32")
    # th staging pre-zeroed (den stripe 1.0) so unwritten halves stay benign
    thstage = const.tile([4, NUNITS, 2, 512], F32, tag="thstage")

    def load_consts():
        nc.sync.dma_start(out=thin4w, in_=io["thin4w"])
        nc.scalar.dma_start(out=iota32, in_=io["iota32"])
        nc.sync.dma_start(out=masks32, in_=io["masks32"])
        nc.scalar.dma_start(out=rlens32, in_=io["rlens32"])
        nc.scalar.dma_start(out=thstage, in_=io["zfill"])

    exp_cycle = [0]

    def exp_step(u, ty, idx, big, dst8, se):
        eng = EXP_PATTERN[exp_cycle[0] % len(EXP_PATTERN)]
        exp_cycle[0] += 1
        bias = u["bias"]
        if eng == "A":
            nc.scalar.activation(dst8, big[:, 0:se], EXP,
                                 bias=bias[:, 0, ty, idx:idx + 1],
                                 scale=float(1.0 / A8))
        else:
            nc.vector.tensor_scalar(out=dst8.bitcast(U8), in0=big[:, 0:se],
                                    scalar1=bias[:, 1, ty, idx:idx + 1],
                                    scalar2=0.0, op0=ALU.add, op1=ALU.max)

    slot_tiles = {}

    def get_slot(s):
        if s in slot_tiles:
            return slot_tiles[s]
        bias = bp.tile([128, 2, 2, NB], F32, tag="bias", name=f"bias{s}")
        nc.sync.dma_start(out=bias, in_=io["bias"][s])
        s1t = s1p.tile([128, 2, 1024], FP8, tag="s1t", name=f"s1t{s}")
        nc.sync.dma_start(out=s1t[:, 0], in_=io["emb1"][s, :, 0])
        nc.scalar.dma_start(out=s1t[:, 1], in_=io["emb1"][s, :, 1])
        dt = dp.tile([128, 2, 1024], FP8, tag="dt", name=f"dt{s}")
        nc.scalar.dma_start(out=dt[:, 0], in_=io["emb2"][s, :, 0])
        nc.sync.dma_start(out=dt[:, 1], in_=io["emb2"][s, :, 1])
        xw = xwp.tile([128, 2, NB, 128], FP8, tag="xw", name=f"xw{s}")
        nc.scalar.dma_start(out=xw[:, 0], in_=io["xw"][s, :, 0])
        nc.sync.dma_start(out=xw[:, 1], in_=io["xw"][s, :, 1])
        t = {"s1c": s1t[:, 0], "s1s": s1t[:, 1],
             "dc": dt[:, 0], "ds": dt[:, 1],
             "xwa": xw[:, 0], "xwb": xw[:, 1], "bias": bias}
        slot_tiles[s] = t
        return t

    def s1_step(u, tb):
        big = ps_big.tile([128, 1024], F32, tag="big", name=f"s1_{u['k']}_{tb}")
        se = u["se"]
        for off, w in _chunks(se):
            nc.tensor.matmul(big[:, off:off + w],
                             lhsT=u["X"][:, 128 * tb:128 * (tb + 1)],
                             rhs=u["Y"][:, off:off + w],
                             start=True, stop=True)
        exp_step(u, u["ty_t"], tb, big, u["et"][:, tb, 0:se], se)

    def s2_c_phase(u, nxt):
        """nn accumulation over j-block pairs (K=256 DoubleRow) into a single
        [128,1024] PSUM tile; row 0 = den. S1 steps of the next unit are
        interleaved to keep the exp engines fed."""
        k, se, Etile = u["k"], u["se"], u["et"]
        npairs = u["nb_t"] // 2
        chunks = _chunks(se)
        n_s1 = nxt["nb_t"] if nxt is not None else 0
        n_res = min(3, n_s1)
        n_pre = n_s1 - n_res
        total_nn = npairs * len(chunks)
        s1_done = 0
        nn_done = 0
        nnp = ps_nn.tile([128, 1024], F32, tag="nn", name=f"nn_{k}")
        for pb in range(npairs):
            for off, w in chunks:
                nc.tensor.matmul(nnp[:, off:off + w],
                                 lhsT=u["xw"][:, 2 * pb:2 * pb + 2, :],
                                 rhs=Etile[:, 2 * pb:2 * pb + 2, off:off + w],
                                 start=pb == 0, stop=pb == npairs - 1,
                                 perf_mode=DR)
                nn_done += 1
                while n_pre and s1_done < (n_pre * nn_done) // total_nn:
                    s1_step(nxt, s1_done)
                    s1_done += 1
        while s1_done < n_pre:
            s1_step(nxt, s1_done)
            s1_done += 1
        rr = rrp.tile([1, 1024], F32, tag="rr")
        nc.vector.reciprocal_approx_fast(out=rr[:, 0:se], in_=nnp[0:1, 0:se])
        bc = bcp.tile([128, 1024], F32, tag="bc")
        nc.gpsimd.partition_broadcast(bc[:, 0:se], rr[:, 0:se])
        nns = nnsp.tile([128, 1024], FP16, tag="nns", name=f"nns_{k}")
        nc.vector.scalar_tensor_tensor(nns[:, 0:se], in0=nnp[:, 0:se],
                                       scalar=1.0, in1=bc[:, 0:se],
                                       op0=ALU.bypass, op1=ALU.mult)
        while s1_done < n_s1:
            s1_step(nxt, s1_done)
            s1_done += 1
        return nns

    def d_step(u, ub):
        se = u["se"]
        big2 = ps_big.tile([128, 1024], F32, tag="big", name=f"d_{u['k']}_{ub}")
        for off, w in _chunks(se):
            nc.tensor.matmul(big2[:, off:off + w],
                             lhsT=u["Dlhs"][:, 128 * ub:128 * (ub + 1)],
                             rhs=u["nns"][:, off:off + w], start=True, stop=True)
        exp_step(u, u["ty_u"], ub, big2, u["bt"][:, ub, 0:se], se)

    def d_flush(u, upb):
        nupairs = u["nb_u"] // 2
        for qi, (off, w) in enumerate(_chunks(u["se"])):
            nc.tensor.matmul(u["th"][0:4, qi, 0:w],
                             lhsT=thin4w[:, 2 * upb:2 * upb + 2, 0:4],
                             rhs=u["bt"][:, 2 * upb:2 * upb + 2, off:off + w],
                             start=upb == 0, stop=upb == nupairs - 1,
                             perf_mode=DR)

    def even(x):
        return x + (x & 1)

    units = []
    for s in range(SLOTS):
        cb, sb = plans[s]
        t = get_slot(s)
        # cycle0: clip->sent->clip; tgt blocks = sent, D/u blocks = clip
        units.append(dict(k=2 * s, nb_t=even(sb), nb_u=even(cb), se=cb * 128,
                          X=t["s1s"], Y=t["s1c"], ty_t=1, ty_u=0,
                          xw=t["xwa"], Dlhs=t["dc"], bias=t["bias"]))
        units.append(dict(k=2 * s + 1, nb_t=even(cb), nb_u=even(sb), se=sb * 128,
                          X=t["s1c"], Y=t["s1s"], ty_t=0, ty_u=1,
                          xw=t["xwb"], Dlhs=t["ds"], bias=t["bias"]))

    load_consts()

    # prologue: S1 of unit 0
    units[0]["et"] = etp.tile([128, NB, 1024], FP8, tag="et", name="et0")
    for tb in range(units[0]["nb_t"]):
        s1_step(units[0], tb)

    acc_tiles = [fin.tile([32, 256], F32, tag=f"acc{c}", name=f"acc{c}")
                 for c in range(4)]

    LAGP = 1
    for j, u in enumerate(units):
        nxt = units[j + 1] if j + 1 < len(units) else None
        if nxt is not None:
            nxt["et"] = etp.tile([128, NB, 1024], FP8, tag="et",
                                 name=f"et{nxt['k']}")
        u["nns"] = s2_c_phase(u, nxt)
        u["bt"] = btp.tile([128, NB, 1024], FP8, tag="bt", name=f"bt{u['k']}")
        u["th"] = ps_nn.tile([4, 2, 512], F32, tag="nn", name=f"th_{u['k']}")
        nupairs = u["nb_u"] // 2
        for ub in range(u["nb_u"]):
            d_step(u, ub)
            if ub % 2 == 1 and (ub // 2) >= LAGP:
                d_flush(u, ub // 2 - LAGP)
        for upb in range(max(0, nupairs - LAGP), nupairs):
            d_flush(u, upb)
        for qi, (off, w) in enumerate(_chunks(u["se"])):
            if TH_COPY_ENGINE == "A":
                nc.scalar.copy(out=thstage[:, u["k"], qi, 0:w],
                               in_=u["th"][:, qi, 0:w])
            else:
                nc.vector.tensor_copy(out=thstage[:, u["k"], qi, 0:w],
                                      in_=u["th"][:, qi, 0:w])
        k = u["k"]
        for c in range(4):
            nc.sync.dma_start(out=acc_tiles[c][4 * k:4 * k + 4, :],
                              in_=thstage[c:c + 1, k, :, :])

    # ---- final: batched loss over [32, 256] rows = (unit, qi, seg) ----
    den32, d1, d2, d3 = acc_tiles
    t4 = fin.tile([32, 256], F32, tag="t4")
    nc.vector.scalar_tensor_tensor(t4, in0=d1, scalar=4.0, in1=d2,
                                   op0=ALU.mult, op1=ALU.add)
    num = fin.tile([32, 256], F32, tag="num")
    nc.vector.scalar_tensor_tensor(num, in0=t4, scalar=16.0, in1=d3,
                                   op0=ALU.mult, op1=ALU.add)
    rden = fin.tile([32, 256], F32, tag="rden")
    scr = fin.tile([32, 256], F32, tag="scr")
    nc.vector.reciprocal_approx_accurate(out=rden, in_=den32, scratch=scr)
    idx = fin.tile([32, 256], F32, tag="idx")
    nc.vector.tensor_mul(idx, num, rden)
    ierr = fin.tile([32, 256], F32, tag="ierr")
    nc.vector.tensor_sub(ierr, idx, iota32)
    tmp = fin.tile([32, 256], F32, tag="tmp")
    nc.vector.tensor_mul(tmp, ierr, masks32)
    sq = fin.tile([32, 256], F32, tag="sq")
    sums = fin.tile([32, 1], F32, tag="sums")
    nc.vector.scalar_tensor_tensor(sq, in0=tmp, scalar=1.0, in1=ierr,
                                   op0=ALU.bypass, op1=ALU.mult, accum_out=sums)
    loss = fin.tile([32, 1], F32, tag="loss")
    nc.vector.tensor_mul(loss, sums, rlens32)
    nc.sync.dma_start(out=io["loss32"], in_=loss)


def _build_program(plans):
    key = tuple(plans)
    if key in _PROGRAM_CACHE:
        return _PROGRAM_CACHE[key]
    nc = bacc.Bacc("TRN2", target_bir_lowering=False, debug=False,
                   num_devices=NCORES)
    io = {
        "emb1": nc.dram_tensor("emb1", [SLOTS, 128, 2, 1024], FP8, kind="ExternalInput").ap(),
        "emb2": nc.dram_tensor("emb2", [SLOTS, 128, 2, 1024], FP8, kind="ExternalInput").ap(),
        "xw": nc.dram_tensor("xw", [SLOTS, 128, 2, NB, 128], FP8, kind="ExternalInput").ap(),
        "bias": nc.dram_tensor("bias", [SLOTS, 128, 2, 2, NB], F32, kind="ExternalInput").ap(),
        "thin4w": nc.dram_tensor("thin4w", [128, NB, 16], FP8, kind="ExternalInput").ap(),
        "iota32": nc.dram_tensor("iota32", [32, 256], F32, kind="ExternalInput").ap(),
        "masks32": nc.dram_tensor("masks32", [32, 256], F32, kind="ExternalInput").ap(),
        "rlens32": nc.dram_tensor("rlens32", [32, 1], F32, kind="ExternalInput").ap(),
        "zfill": nc.dram_tensor("zfill", [4, NUNITS, 2, 512], F32, kind="ExternalInput").ap(),
        "loss32": nc.dram_tensor("loss32", [32, 1], F32, kind="ExternalOutput").ap(),
    }
    from contextlib import ExitStack
    with tile.TileContext(nc) as tc:
        with ExitStack() as ctx:
            _emit(nc, tc, ctx, io, plans)
    nc.compile()
    _PROGRAM_CACHE[key] = nc
    return nc


def _q8(x):
    return np.asarray(x, np.float32).astype(E4)


def _host_prep(clip_emb, clip_mask, clip_lens, sent_emb, sent_mask, sent_lens):
    """Sorted batch->(core,slot) assignment, per-slot plans, per-core inputs."""
    cb_all = np.ceil(clip_lens / 128).astype(int)
    sb_all = np.ceil(sent_lens / 128).astype(int)
    order = np.argsort(-(cb_all + sb_all) * 1000 - cb_all)  # big batches first
    plans = []
    assign = {}
    for s in range(SLOTS):
        grp = order[8 * s:8 * s + 8]
        plans.append((int(cb_all[grp].max()), int(sb_all[grp].max())))
        for core, b in enumerate(grp):
            assign[(core, s)] = int(b)

    sq_c = np.einsum("bmd,bmd->bm", clip_emb, clip_emb)
    sq_s = np.einsum("bnd,bnd->bn", sent_emb, sent_emb)
    bias_c = (-sq_c / D + PEN * (1.0 - clip_mask)).astype(np.float32)
    bias_s = (-sq_s / D + PEN * (1.0 - sent_mask)).astype(np.float32)

    u = np.arange(M)
    thin4w = np.zeros((128, NB, 16), E4)
    for ub in range(NB):
        uu = u[ub * 128:(ub + 1) * 128]
        thin4w[:, ub, 0] = 1.0
        thin4w[:, ub, 1] = (uu >> 6).astype(np.float32)
        thin4w[:, ub, 2] = ((uu >> 4) & 3).astype(np.float32)
        thin4w[:, ub, 3] = (uu & 15).astype(np.float32)

    iota32 = np.arange(M, dtype=np.float32).reshape(1, 4, 256)
    iota32 = np.broadcast_to(iota32, (NUNITS, 4, 256)).reshape(32, 256).copy()
    zfill = np.zeros((4, NUNITS, 2, 512), np.float32)
    zfill[0] = 1.0

    in_maps = []
    for core in range(NCORES):
        bs = [assign[(core, s)] for s in range(SLOTS)]
        emb1 = np.zeros((SLOTS, 128, 2, 1024), E4)
        emb2 = np.zeros((SLOTS, 128, 2, 1024), E4)
        xw = np.zeros((SLOTS, 128, 2, NB, 128), E4)
        bias = np.zeros((SLOTS, 128, 2, 2, NB), np.float32)
        masks32 = np.zeros((32, 256), np.float32)
        rlens32 = np.zeros((32, 1), np.float32)
        for s, b in enumerate(bs):
            c = clip_emb[b].astype(np.float32)   # [M, D]
            t = sent_emb[b].astype(np.float32)   # [N, D]
            # S1 operands: [p(64), kt, i] = emb[i, kt*64+p] * ALPHA1
            cs = _q8(c * ALPHA1)  # [M, D]
            ts = _q8(t * ALPHA1)
            emb1[s, :, 0] = cs.T
            emb1[s, :, 1] = ts.T
            # D lhsT fp8: row0 = 0, rows 1..127 = emb[u, e-1]*ALPHA2
            c2 = np.zeros((128, M), np.float32)
            c2[1:, :] = _q8(c[:, :127] * ALPHA2).astype(np.float32).T
            t2 = np.zeros((128, N), np.float32)
            t2[1:, :] = _q8(t[:, :127] * ALPHA2).astype(np.float32).T
            emb2[s, :, 0] = c2.astype(E4)
            emb2[s, :, 1] = t2.astype(E4)
            # S2 lhsT: [p, tb, e]: e=0 -> 1 else tgt[tb*128+p, e-1]
            xa = np.zeros((128, NB, 128), np.float32)
            xa[:, :, 0] = 1.0
            xa[:, :, 1:] = _q8(t[:, :127]).astype(np.float32).reshape(NB, 128, 127).transpose(1, 0, 2)
            xb = np.zeros((128, NB, 128), np.float32)
            xb[:, :, 0] = 1.0
            xb[:, :, 1:] = _q8(c[:, :127]).astype(np.float32).reshape(NB, 128, 127).transpose(1, 0, 2)
            xw[s, :, 0] = xa.astype(E4)
            xw[s, :, 1] = xb.astype(E4)
            # biases: [p, btype(exact,bits), which(c,s), tb]
            bias[s, :, 0, 0] = bias_c[b].reshape(NB, 128).T
            bias[s, :, 0, 1] = bias_s[b].reshape(NB, 128).T
            bias[s, :, 1, 0] = (A8 * bias_c[b] + MAGIC).reshape(NB, 128).T
            bias[s, :, 1, 1] = (A8 * bias_s[b] + MAGIC).reshape(NB, 128).T
            # final-phase rows r = 4k + 2*qi + seg
            for cyc, (msk, ln) in enumerate(((clip_mask[b], clip_lens[b]),
                                             (sent_mask[b], sent_lens[b]))):
                k = 2 * s + cyc
                masks32[4 * k:4 * k + 4] = msk.reshape(4, 256)
                rlens32[4 * k:4 * k + 4] = 1.0 / ln
        in_maps.append({
            "emb1": emb1, "emb2": emb2, "xw": xw, "bias": bias,
            "thin4w": thin4w, "iota32": iota32, "masks32": masks32,
            "rlens32": rlens32, "zfill": zfill,
        })
    return in_maps, assign, plans


def kernel(clip_emb, clip_mask, clip_lens, sent_emb, sent_mask, sent_lens):
    global LAST_RESULT
    clip_emb = np.asarray(clip_emb, np.float32)
    sent_emb = np.asarray(sent_emb, np.float32)
    clip_mask = np.asarray(clip_mask, np.float32)
    sent_mask = np.asarray(sent_mask, np.float32)
    clip_lens = np.asarray(clip_lens, np.float32)
    sent_lens = np.asarray(sent_lens, np.float32)

    in_maps, _, plans = _host_prep(clip_emb, clip_mask, clip_lens,
                                   sent_emb, sent_mask, sent_lens)
    nc = _build_program(plans)
    res = run_bass_kernel_spmd(nc, in_maps, list(range(NCORES)))
    LAST_RESULT = res

    rows = np.stack([res.results[c]["loss32"].reshape(32) for c in range(NCORES)])
    per_unit = rows.reshape(NCORES, NUNITS, 4).sum(axis=2)
    clip_loss = per_unit[:, 0::2].mean()
    sent_loss = per_unit[:, 1::2].mean()
    return (np.float32(clip_loss), np.float32(sent_loss))


# revision 12
# speedup vs baseline: 1.0666x; 1.0039x over previous
"""CycleConsistencyLoss on 8 Trainium2 NeuronCores (Bass/Tile, SPMD data-parallel).

Math (per batch, clip [M,D], sent [N,D], prefix masks):
  soft_nn(src,tgt): w = softmax_j(-dist(src_i,tgt_j) masked); nn = w @ tgt
  dist = (|s|^2+|t|^2-2 s.t)/D; softmax shift-invariance =>
  w[i,j] prop exp((2 s_i.t_j - |t_j|^2)/D) * mask_j
  index_nn = sum_u u*beta / sum_u beta over tgt2 = src embeddings
  loss_c = mean_b sum_i (index_nn[i]-i)^2 * mask_i / len_b

fp8 e4m3 design (validated vs f64 reference at ~1e-5 rel err):
  All embedding operands pre-quantized e4m3 on host; matmuls use fp8
  DoubleRow perf mode where the layout allows:
   S1  scores: normal-mode fp8 [128,*] (LD hides under stream)
   S2  nn acc: lhsT [128,2,128] over j-block PAIRS (K=256)    -> 0.25
   D   beta:   normal-mode fp8 [128,*]                        -> 1.0
   FL  den/num: lhsT [128,2,4] over u-block pairs             -> 0.25-ish
  index weights for FL are digit-split u = 64*d1 + 16*d2 + d3 (each digit
  e4m3-exact); recombined in f32 in the final phase.
  exp runs on TWO engines, split per score tile:
   ACT: exp activation, bias=-|t|^2/D(+pen), scale=1/A8, fp8 out
   DVE: Schraudolph bit-hack: uint8 = max(psum + biasbits, 0) where psum is
        pre-scaled by A8*2/D via the host embedding scaling; uint8 bit
        pattern IS the e4m3 weight. Masked rows -> bits<0 -> clamp 0.
  Odd block counts are padded to even: the pad block's scores get the
  masked-row bias (pen) so its weights underflow to exactly 0 in fp8.
  th staging is pre-zeroed via a DMA'd constant (den stripe=1.0) so units
  with se<=512 leave benign values in the unused half.
"""
import sys

sys.path.insert(0, "/opt/trn_rl_repo")

import numpy as np
import ml_dtypes

import concourse.bass as bass
import concourse.tile as tile
from concourse import bacc, mybir
from concourse.bass_utils import run_bass_kernel_spmd

F32 = mybir.dt.float32
FP8 = mybir.dt.float8e4
U8 = mybir.dt.uint8
EXP = mybir.ActivationFunctionType.Exp
ALU = mybir.AluOpType
DR = mybir.MatmulPerfMode.DoubleRow
FP16 = mybir.dt.float16

B, M, N, D = 32, 1024, 1024, 128
NB = M // 128
NCORES = 8
SLOTS = B // NCORES  # 4
NUNITS = 2 * SLOTS
PEN = -20.0
A8 = 8.0 / np.log(2.0)       # e4m3 bits per e-fold
MAGIC = 56.5                  # 7*8 exponent bias + 0.5 round
ALPHA1 = np.sqrt(2.0 * A8 / D)  # S1 per-side embedding scale
ALPHA2 = 2.0 * A8 / D           # D-stage lhsT scale
E4 = ml_dtypes.float8_e4m3

# exp engine split: 'A' = ACT exact exp, 'D' = DVE bit-hack
EXP_PATTERN = "ADADADA"
TH_COPY_ENGINE = "A"

_PROGRAM_CACHE = {}
LAST_RESULT = None


def _chunks(ext):
    if ext <= 512:
        return [(0, ext)]
    return [(0, 512), (512, ext - 512)]


def _emit(nc, tc, ctx, io, plans):
    const = ctx.enter_context(tc.tile_pool(name="const", bufs=1))
    s1p = ctx.enter_context(tc.tile_pool(name="s1p", bufs=2))
    dp = ctx.enter_context(tc.tile_pool(name="dp", bufs=2))
    xwp = ctx.enter_context(tc.tile_pool(name="xwp", bufs=2))
    bp = ctx.enter_context(tc.tile_pool(name="bp", bufs=2))
    etp = ctx.enter_context(tc.tile_pool(name="etp", bufs=2))
    btp = ctx.enter_context(tc.tile_pool(name="btp", bufs=2))
    nnsp = ctx.enter_context(tc.tile_pool(name="nnsp", bufs=2))
    rrp = ctx.enter_context(tc.tile_pool(name="rrp", bufs=2))
    bcp = ctx.enter_context(tc.tile_pool(name="bcp", bufs=2))
    fin = ctx.enter_context(tc.tile_pool(name="fin", bufs=1))

    ps_big = ctx.enter_context(tc.tile_pool(name="ps_big", bufs=3, space="PSUM"))
    ps_nn = ctx.enter_context(tc.tile_pool(name="ps_nn", bufs=1, space="PSUM"))

    thin4w = const.tile([128, NB, 16], FP8, tag="thin4w")
    iota32 = const.tile([32, 256], F32, tag="iota32")
    masks32 = const.tile([32, 256], F32, tag="masks32")
    rlens32 = const.tile([32, 1], F32, tag="rlens32")
    # th staging pre-zeroed (den stripe 1.0) so unwritten halves stay benign
    thstage = const.tile([4, NUNITS, 2, 512], F32, tag="thstage")

    def load_consts():
        nc.sync.dma_start(out=thin4w, in_=io["thin4w"])
        nc.scalar.dma_start(out=iota32, in_=io["iota32"])
        nc.sync.dma_start(out=masks32, in_=io["masks32"])
        nc.scalar.dma_start(out=rlens32, in_=io["rlens32"])
        nc.scalar.dma_start(out=thstage, in_=io["zfill"])

    exp_cycle = [0]

    def exp_step(u, ty, idx, big, dst8, se):
        eng = EXP_PATTERN[exp_cycle[0] % len(EXP_PATTERN)]
        exp_cycle[0] += 1
        bias = u["bias"]
        if eng == "A":
            nc.scalar.activation(dst8, big[:, 0:se], EXP,
                                 bias=bias[:, 0, ty, idx:idx + 1],
                                 scale=float(1.0 / A8))
        else:
            nc.vector.tensor_scalar(out=dst8.bitcast(U8), in0=big[:, 0:se],
                                    scalar1=bias[:, 1, ty, idx:idx + 1],
                                    scalar2=0.0, op0=ALU.add, op1=ALU.max)

    slot_tiles = {}

    def get_slot(s):
        if s in slot_tiles:
            return slot_tiles[s]
        bias = bp.tile([128, 2, 2, NB], F32, tag="bias", name=f"bias{s}")
        nc.sync.dma_start(out=bias, in_=io["bias"][s])
        s1t = s1p.tile([128, 2, 1024], FP8, tag="s1t", name=f"s1t{s}")
        # lhsT (sent) head + rhs (clip) first chunk land first
        nc.scalar.dma_start(out=s1t[:, 1, 0:256], in_=io["emb1"][s, :, 1, 0:256])
        nc.sync.dma_start(out=s1t[:, 0, 0:512], in_=io["emb1"][s, :, 0, 0:512])
        nc.scalar.dma_start(out=s1t[:, 1, 256:], in_=io["emb1"][s, :, 1, 256:])
        nc.sync.dma_start(out=s1t[:, 0, 512:], in_=io["emb1"][s, :, 0, 512:])
        xw = xwp.tile([128, 2, NB, 128], FP8, tag="xw", name=f"xw{s}")
        nc.scalar.dma_start(out=xw[:, 0], in_=io["xw"][s, :, 0])
        nc.sync.dma_start(out=xw[:, 1], in_=io["xw"][s, :, 1])
        dt = dp.tile([128, 2, 1024], FP8, tag="dt", name=f"dt{s}")
        nc.scalar.dma_start(out=dt[:, 0], in_=io["emb2"][s, :, 0])
        nc.sync.dma_start(out=dt[:, 1], in_=io["emb2"][s, :, 1])
        t = {"s1c": s1t[:, 0], "s1s": s1t[:, 1],
             "dc": dt[:, 0], "ds": dt[:, 1],
             "xwa": xw[:, 0], "xwb": xw[:, 1], "bias": bias}
        slot_tiles[s] = t
        return t

    def s1_step(u, tb):
        big = ps_big.tile([128, 1024], F32, tag="big", name=f"s1_{u['k']}_{tb}")
        se = u["se"]
        for off, w in _chunks(se):
            nc.tensor.matmul(big[:, off:off + w],
                             lhsT=u["X"][:, 128 * tb:128 * (tb + 1)],
                             rhs=u["Y"][:, off:off + w],
                             start=True, stop=True)
        exp_step(u, u["ty_t"], tb, big, u["et"][:, tb, 0:se], se)

    def s2_c_phase(u, nxt):
        """nn accumulation over j-block pairs (K=256 DoubleRow) into a single
        [128,1024] PSUM tile; row 0 = den. S1 steps of the next unit are
        interleaved to keep the exp engines fed."""
        k, se, Etile = u["k"], u["se"], u["et"]
        npairs = u["nb_t"] // 2
        chunks = _chunks(se)
        n_s1 = nxt["nb_t"] if nxt is not None else 0
        n_res = min(3, n_s1)
        n_pre = n_s1 - n_res
        total_nn = npairs * len(chunks)
        s1_done = 0
        nn_done = 0
        nnp = ps_nn.tile([128, 1024], F32, tag="nn", name=f"nn_{k}")
        for pb in range(npairs):
            for off, w in chunks:
                nc.tensor.matmul(nnp[:, off:off + w],
                                 lhsT=u["xw"][:, 2 * pb:2 * pb + 2, :],
                                 rhs=Etile[:, 2 * pb:2 * pb + 2, off:off + w],
                                 start=pb == 0, stop=pb == npairs - 1,
                                 perf_mode=DR)
                nn_done += 1
                while n_pre and s1_done < (n_pre * nn_done) // total_nn:
                    s1_step(nxt, s1_done)
                    s1_done += 1
        while s1_done < n_pre:
            s1_step(nxt, s1_done)
            s1_done += 1
        rr = rrp.tile([1, 1024], F32, tag="rr")
        nc.vector.reciprocal_approx_fast(out=rr[:, 0:se], in_=nnp[0:1, 0:se])
        bc = bcp.tile([128, 1024], F32, tag="bc")
        nc.gpsimd.partition_broadcast(bc[:, 0:se], rr[:, 0:se])
        nns = nnsp.tile([128, 1024], FP8, tag="nns", name=f"nns_{k}")
        nc.vector.scalar_tensor_tensor(nns[:, 0:se], in0=nnp[:, 0:se],
                                       scalar=1.0, in1=bc[:, 0:se],
                                       op0=ALU.bypass, op1=ALU.mult)
        while s1_done < n_s1:
            s1_step(nxt, s1_done)
            s1_done += 1
        return nns

    def d_step(u, ub):
        se = u["se"]
        big2 = ps_big.tile([128, 1024], F32, tag="big", name=f"d_{u['k']}_{ub}")
        for off, w in _chunks(se):
            nc.tensor.matmul(big2[:, off:off + w],
                             lhsT=u["Dlhs"][:, 128 * ub:128 * (ub + 1)],
                             rhs=u["nns"][:, off:off + w], start=True, stop=True)
        exp_step(u, u["ty_u"], ub, big2, u["bt"][:, ub, 0:se], se)

    def d_flush(u, upb):
        nupairs = u["nb_u"] // 2
        for qi, (off, w) in enumerate(_chunks(u["se"])):
            nc.tensor.matmul(u["th"][0:4, qi, 0:w],
                             lhsT=thin4w[:, 2 * upb:2 * upb + 2, 0:4],
                             rhs=u["bt"][:, 2 * upb:2 * upb + 2, off:off + w],
                             start=upb == 0, stop=upb == nupairs - 1,
                             perf_mode=DR)

    def even(x):
        return x + (x & 1)

    units = []
    for s in range(SLOTS):
        cb, sb = plans[s]
        t = get_slot(s)
        # cycle0: clip->sent->clip; tgt blocks = sent, D/u blocks = clip
        units.append(dict(k=2 * s, nb_t=even(sb), nb_u=even(cb), se=cb * 128,
                          X=t["s1s"], Y=t["s1c"], ty_t=1, ty_u=0,
                          xw=t["xwa"], Dlhs=t["dc"], bias=t["bias"]))
        units.append(dict(k=2 * s + 1, nb_t=even(cb), nb_u=even(sb), se=sb * 128,
                          X=t["s1c"], Y=t["s1s"], ty_t=0, ty_u=1,
                          xw=t["xwb"], Dlhs=t["ds"], bias=t["bias"]))

    load_consts()

    # prologue: S1 of unit 0
    units[0]["et"] = etp.tile([128, NB, 1024], FP8, tag="et", name="et0")
    for tb in range(units[0]["nb_t"]):
        s1_step(units[0], tb)

    acc_tiles = [fin.tile([32, 256], F32, tag=f"acc{c}", name=f"acc{c}")
                 for c in range(4)]

    LAGP = 1
    for j, u in enumerate(units):
        nxt = units[j + 1] if j + 1 < len(units) else None
        if nxt is not None:
            nxt["et"] = etp.tile([128, NB, 1024], FP8, tag="et",
                                 name=f"et{nxt['k']}")
        u["nns"] = s2_c_phase(u, nxt)
        u["bt"] = btp.tile([128, NB, 1024], FP8, tag="bt", name=f"bt{u['k']}")
        u["th"] = ps_nn.tile([4, 2, 512], F32, tag="nn", name=f"th_{u['k']}")
        nupairs = u["nb_u"] // 2
        for ub in range(u["nb_u"]):
            d_step(u, ub)
            if ub % 2 == 1 and (ub // 2) >= LAGP:
                d_flush(u, ub // 2 - LAGP)
        for upb in range(max(0, nupairs - LAGP), nupairs):
            d_flush(u, upb)
        for qi, (off, w) in enumerate(_chunks(u["se"])):
            if TH_COPY_ENGINE == "A":
                nc.scalar.copy(out=thstage[:, u["k"], qi, 0:w],
                               in_=u["th"][:, qi, 0:w])
            else:
                nc.vector.tensor_copy(out=thstage[:, u["k"], qi, 0:w],
                                      in_=u["th"][:, qi, 0:w])
        k = u["k"]
        for c in range(4):
            eng = nc.sync if c % 2 == 0 else nc.scalar
            eng.dma_start(out=acc_tiles[c][4 * k:4 * k + 4, :],
                          in_=thstage[c:c + 1, k, :, :])

    # ---- final: batched loss over [32, 256] rows = (unit, qi, seg) ----
    den32, d1, d2, d3 = acc_tiles
    t4 = fin.tile([32, 256], F32, tag="t4")
    nc.vector.scalar_tensor_tensor(t4, in0=d1, scalar=4.0, in1=d2,
                                   op0=ALU.mult, op1=ALU.add)
    num = fin.tile([32, 256], F32, tag="num")
    nc.vector.scalar_tensor_tensor(num, in0=t4, scalar=16.0, in1=d3,
                                   op0=ALU.mult, op1=ALU.add)
    rden = fin.tile([32, 256], F32, tag="rden")
    scr = fin.tile([32, 256], F32, tag="scr")
    nc.vector.reciprocal_approx_accurate(out=rden, in_=den32, scratch=scr)
    idx = fin.tile([32, 256], F32, tag="idx")
    nc.vector.tensor_mul(idx, num, rden)
    ierr = fin.tile([32, 256], F32, tag="ierr")
    nc.vector.tensor_sub(ierr, idx, iota32)
    tmp = fin.tile([32, 256], F32, tag="tmp")
    nc.vector.tensor_mul(tmp, ierr, masks32)
    sq = fin.tile([32, 256], F32, tag="sq")
    sums = fin.tile([32, 1], F32, tag="sums")
    nc.vector.scalar_tensor_tensor(sq, in0=tmp, scalar=1.0, in1=ierr,
                                   op0=ALU.bypass, op1=ALU.mult, accum_out=sums)
    loss = fin.tile([32, 1], F32, tag="loss")
    nc.vector.tensor_mul(loss, sums, rlens32)
    nc.sync.dma_start(out=io["loss32"], in_=loss)


def _build_program(plans):
    key = tuple(plans)
    if key in _PROGRAM_CACHE:
        return _PROGRAM_CACHE[key]
    nc = bacc.Bacc("TRN2", target_bir_lowering=False, debug=False,
                   num_devices=NCORES)
    io = {
        "emb1": nc.dram_tensor("emb1", [SLOTS, 128, 2, 1024], FP8, kind="ExternalInput").ap(),
        "emb2": nc.dram_tensor("emb2", [SLOTS, 128, 2, 1024], FP8, kind="ExternalInput").ap(),
        "xw": nc.dram_tensor("xw", [SLOTS, 128, 2, NB, 128], FP8, kind="ExternalInput").ap(),
        "bias": nc.dram_tensor("bias", [SLOTS, 128, 2, 2, NB], F32, kind="ExternalInput").ap(),
        "thin4w": nc.dram_tensor("thin4w", [128, NB, 16], FP8, kind="ExternalInput").ap(),
        "iota32": nc.dram_tensor("iota32", [32, 256], F32, kind="ExternalInput").ap(),
        "masks32": nc.dram_tensor("masks32", [32, 256], F32, kind="ExternalInput").ap(),
        "rlens32": nc.dram_tensor("rlens32", [32, 1], F32, kind="ExternalInput").ap(),
        "zfill": nc.dram_tensor("zfill", [4, NUNITS, 2, 512], F32, kind="ExternalInput").ap(),
        "loss32": nc.dram_tensor("loss32", [32, 1], F32, kind="ExternalOutput").ap(),
    }
    from contextlib import ExitStack
    with tile.TileContext(nc) as tc:
        with ExitStack() as ctx:
            _emit(nc, tc, ctx, io, plans)
    nc.compile()
    _PROGRAM_CACHE[key] = nc
    return nc


def _q8(x):
    return np.asarray(x, np.float32).astype(E4)


def _host_prep(clip_emb, clip_mask, clip_lens, sent_emb, sent_mask, sent_lens):
    """Sorted batch->(core,slot) assignment, per-slot plans, per-core inputs."""
    cb_all = np.ceil(clip_lens / 128).astype(int)
    sb_all = np.ceil(sent_lens / 128).astype(int)
    order = np.argsort(-(cb_all + sb_all) * 1000 - cb_all)  # big batches first
    plans = []
    assign = {}
    for s in range(SLOTS):
        grp = order[8 * s:8 * s + 8]
        plans.append((int(cb_all[grp].max()), int(sb_all[grp].max())))
        for core, b in enumerate(grp):
            assign[(core, s)] = int(b)

    sq_c = np.einsum("bmd,bmd->bm", clip_emb, clip_emb)
    sq_s = np.einsum("bnd,bnd->bn", sent_emb, sent_emb)
    bias_c = (-sq_c / D + PEN * (1.0 - clip_mask)).astype(np.float32)
    bias_s = (-sq_s / D + PEN * (1.0 - sent_mask)).astype(np.float32)

    u = np.arange(M)
    thin4w = np.zeros((128, NB, 16), E4)
    for ub in range(NB):
        uu = u[ub * 128:(ub + 1) * 128]
        thin4w[:, ub, 0] = 1.0
        thin4w[:, ub, 1] = (uu >> 6).astype(np.float32)
        thin4w[:, ub, 2] = ((uu >> 4) & 3).astype(np.float32)
        thin4w[:, ub, 3] = (uu & 15).astype(np.float32)

    iota32 = np.arange(M, dtype=np.float32).reshape(1, 4, 256)
    iota32 = np.broadcast_to(iota32, (NUNITS, 4, 256)).reshape(32, 256).copy()
    zfill = np.zeros((4, NUNITS, 2, 512), np.float32)
    zfill[0] = 1.0

    in_maps = []
    for core in range(NCORES):
        bs = [assign[(core, s)] for s in range(SLOTS)]
        emb1 = np.zeros((SLOTS, 128, 2, 1024), E4)
        emb2 = np.zeros((SLOTS, 128, 2, 1024), E4)
        xw = np.zeros((SLOTS, 128, 2, NB, 128), E4)
        bias = np.zeros((SLOTS, 128, 2, 2, NB), np.float32)
        masks32 = np.zeros((32, 256), np.float32)
        rlens32 = np.zeros((32, 1), np.float32)
        for s, b in enumerate(bs):
            c = clip_emb[b].astype(np.float32)   # [M, D]
            t = sent_emb[b].astype(np.float32)   # [N, D]
            # S1 operands: [p(64), kt, i] = emb[i, kt*64+p] * ALPHA1
            cs = _q8(c * ALPHA1)  # [M, D]
            ts = _q8(t * ALPHA1)
            emb1[s, :, 0] = cs.T
            emb1[s, :, 1] = ts.T
            # D lhsT fp8: row0 = 0, rows 1..127 = emb[u, e-1]*ALPHA2
            c2 = np.zeros((128, M), np.float32)
            c2[1:, :] = _q8(c[:, :127] * ALPHA2).astype(np.float32).T
            t2 = np.zeros((128, N), np.float32)
            t2[1:, :] = _q8(t[:, :127] * ALPHA2).astype(np.float32).T
            emb2[s, :, 0] = c2.astype(E4)
            emb2[s, :, 1] = t2.astype(E4)
            # S2 lhsT: [p, tb, e]: e=0 -> 1 else tgt[tb*128+p, e-1]
            xa = np.zeros((128, NB, 128), np.float32)
            xa[:, :, 0] = 1.0
            xa[:, :, 1:] = _q8(t[:, :127]).astype(np.float32).reshape(NB, 128, 127).transpose(1, 0, 2)
            xb = np.zeros((128, NB, 128), np.float32)
            xb[:, :, 0] = 1.0
            xb[:, :, 1:] = _q8(c[:, :127]).astype(np.float32).reshape(NB, 128, 127).transpose(1, 0, 2)
            xw[s, :, 0] = xa.astype(E4)
            xw[s, :, 1] = xb.astype(E4)
            # biases: [p, btype(exact,bits), which(c,s), tb]
            bias[s, :, 0, 0] = bias_c[b].reshape(NB, 128).T
            bias[s, :, 0, 1] = bias_s[b].reshape(NB, 128).T
            bias[s, :, 1, 0] = (A8 * bias_c[b] + MAGIC).reshape(NB, 128).T
            bias[s, :, 1, 1] = (A8 * bias_s[b] + MAGIC).reshape(NB, 128).T
            # final-phase rows r = 4k + 2*qi + seg
            for cyc, (msk, ln) in enumerate(((clip_mask[b], clip_lens[b]),
                                             (sent_mask[b], sent_lens[b]))):
                k = 2 * s + cyc
                masks32[4 * k:4 * k + 4] = msk.reshape(4, 256)
                rlens32[4 * k:4 * k + 4] = 1.0 / ln
        in_maps.append({
            "emb1": emb1, "emb2": emb2, "xw": xw, "bias": bias,
            "thin4w": thin4w, "iota32": iota32, "masks32": masks32,
            "rlens32": rlens32, "zfill": zfill,
        })
    return in_maps, assign, plans


def kernel(clip_emb, clip_mask, clip_lens, sent_emb, sent_mask, sent_lens):
    global LAST_RESULT
    clip_emb = np.asarray(clip_emb, np.float32)
    sent_emb = np.asarray(sent_emb, np.float32)
    clip_mask = np.asarray(clip_mask, np.float32)
    sent_mask = np.asarray(sent_mask, np.float32)
    clip_lens = np.asarray(clip_lens, np.float32)
    sent_lens = np.asarray(sent_lens, np.float32)

    in_maps, _, plans = _host_prep(clip_emb, clip_mask, clip_lens,
                                   sent_emb, sent_mask, sent_lens)
    nc = _build_program(plans)
    res = run_bass_kernel_spmd(nc, in_maps, list(range(NCORES)))
    LAST_RESULT = res

    rows = np.stack([res.results[c]["loss32"].reshape(32) for c in range(NCORES)])
    per_unit = rows.reshape(NCORES, NUNITS, 4).sum(axis=2)
    clip_loss = per_unit[:, 0::2].mean()
    sent_loss = per_unit[:, 1::2].mean()
    return (np.float32(clip_loss), np.float32(sent_loss))
